# revision 35
# baseline (speedup 1.0000x reference)
"""MinGRU on Trainium2 (Bass/Tile), data-parallel over batch on 8 NeuronCores.

Math (per batch element, per hidden channel):
    k_z = x @ W_z.T + b_z
    k_h = x @ W_h.T + b_h
    a   = sigmoid(-k_z)                  # = exp(log_coeffs)
    z   = sigmoid(k_z) = 1 - a
    g(u)= u + 0.5 if u >= 0 else sigmoid(u)
        = max(u + 0.5, sigmoid(u))       # exact: sigmoid(u)-(u+0.5) >=0 iff u<=0
    v   = z * g(k_h)
    h_t = a_t * h_{t-1} + v_t,  h_init = g(h_0)        (t = 1..T)
Output is h_1..h_T, shape [B, T, H].

Device layout: each core gets one batch element. Hidden dim H on SBUF
partitions (8 tiles of 128), time T on the free dim; the recurrence is the
DVE TensorTensorScan instruction (fp32 state).

Precision (validated by CPU emulation, rel_err 1.57e-2 vs 2e-2 gate):
  - One fp8(e4m3) x stream at x/8 scale feeds BOTH matmuls' fp8 parts
    (fp8 products need consistent PSUM scale; w carries the complement).
  - k_z: all 4 DoubleRow pairs fp8 (w = 4096*W_z, descale 2^-9 folded into
    the ACT scale). k_z errors reach h only through sigmoid' <= 1/4.
  - k_h: 1 DoubleRow pair fp8 (rows 0:255, w = 8*W_h, true scale) + 6 fp16
    i-tiles (rows 256:1023). g has slope 1, so k_h tolerates only ~1/4 of
    the contraction in fp8. fp16 tiles (not bf16) keep the pipeline's own
    rounding noise negligible.
  - Tail per tile: ACT a = sig(-2^-9 kz - bz), z = sig(+2^-9 kz + bz),
    s = sig(kh + bh) (3 ACT ops from PSUM); DVE g = (kh +(bh+.5)) max s
    (one STT from PSUM), v = z*g (2x TT), h = scan(a, v). Everything fp16.
  - PE/tile(1024): kz 8 MM + kh 14 MM = 22 x 216ns -> ~152us busy; DVE
    ~128us, ACT ~92us fit underneath.

DMA: weight pair loads split across 4 rings (gpsimd/sync/scalar/vector) so
the first k_z matmul waits on ~2 transfers instead of a serial ring. x8
chunks on gpsimd, x16 on sync, stores alternate sync/gpsimd. Chunks
[512,1024,1024,1024,256,256]: small head chunk to start the PE early,
small tail chunks so the last scan+store tail is short.
"""

import numpy as np
from contextlib import ExitStack

import concourse.bass as bass
import concourse.tile as tile
from concourse import bacc, mybir
from concourse.bass_utils import run_bass_kernel_spmd

B, T, I, H = 8, 4096, 1024, 1024
P = 128           # SBUF partitions
TC = 1024         # max T chunk (2 PSUM banks of fp32)
MN = 512          # matmul moving free dim (one PSUM bank)
CHUNKS = [1024, 1024, 1024, 512, 256, 256]
assert sum(CHUNKS) == T
NI, NH = I // P, H // P
NQ = NI // 2      # fp8 DoubleRow contraction pairs (kz)
KH8Q = 1          # kh fp8 DoubleRow pairs (rows 0 : KH8Q*256)
NI16 = NI - 2 * KH8Q   # kh fp16 i-tiles (rows KH8Q*256 : 1024)
NT = len(CHUNKS)
F32 = mybir.dt.float32
FP16 = mybir.dt.float16
FP8 = mybir.dt.float8e4
import ml_dtypes
FP8_NP = ml_dtypes.float8_e4m3
XS = 0.125        # host-side fp8 scale on x (shared by kz and kh fp8 parts)
WZS = 4096.0      # host-side fp8 scale on W_z
WHS = 8.0         # host-side fp8 scale on W_h pair (XS*WHS = 1: true scale)
DESCALE_Z = 1.0 / (XS * WZS)  # 2^-9
AF = mybir.ActivationFunctionType
OP = mybir.AluOpType
DR = mybir.MatmulPerfMode.DoubleRow

_PROGRAM = None


def _build_program():
    nc = bacc.Bacc("TRN2", target_bir_lowering=False, debug=False)
    xT = nc.dram_tensor("xT", [NI16 * P, T], FP16, kind="ExternalInput").ap()
    x8 = nc.dram_tensor("x8", [I, T], FP8, kind="ExternalInput").ap()
    wzT = nc.dram_tensor("wzT", [I, H], FP8, kind="ExternalInput").ap()
    wh8T = nc.dram_tensor("wh8T", [2 * KH8Q * P, H], FP8,
                          kind="ExternalInput").ap()
    whT = nc.dram_tensor("whT", [NI16 * P, H], FP16, kind="ExternalInput").ap()
    # packed per-partition-contiguous consts: [-b_z, b_z, b_h, b_h+0.5, h_0]
    consts = nc.dram_tensor("consts", [5 * H], F32, kind="ExternalInput").ap()
    out = nc.dram_tensor("out", [H, T], FP16, kind="ExternalOutput").ap()

    with tile.TileContext(nc) as tc, ExitStack() as ctx:
        const = ctx.enter_context(tc.tile_pool(name="const", bufs=1))
        xpool = ctx.enter_context(tc.tile_pool(name="xp", bufs=2))
        spool = ctx.enter_context(tc.tile_pool(name="xs", bufs=3))
        psum = ctx.enter_context(tc.tile_pool(name="ps", bufs=2, space="PSUM"))
        apool = ctx.enter_context(tc.tile_pool(name="ap", bufs=10))
        act = ctx.enter_context(tc.tile_pool(name="actp", bufs=4))
        hpool = ctx.enter_context(tc.tile_pool(name="hp", bufs=2))

        wzT_r = wzT.rearrange("(k p) h -> p k h", p=P)
        wh8T_r = wh8T.rearrange("(k p) h -> p k h", p=P)
        whT_r = whT.rearrange("(n p) h -> p n h", p=P)
        xT_r = xT.rearrange("(n p) t -> p n t", p=P)
        x8_r = x8.rearrange("(k p) t -> p k t", p=P)

        x8_tiles = [[None] * NQ for _ in range(NT)]
        x16_tiles = [[None] * NI16 for _ in range(NT)]
        h_tiles = [[None] * NH for _ in range(NT)]

        # PE warmup: throwaway matmuls on memset tiles while the first DMAs
        # land; ~4us of sustained PE activity moves the HAM clock-gate from
        # 4/8 to 8/8 so the first real matmuls run at 2.4 GHz.
        warm_w = const.tile([P, P], FP16, tag="warmw", name="warm_w")
        warm_x = const.tile([P, MN], FP16, tag="warmx", name="warm_x")
        nc.vector.memset(warm_w[:], 0.0)
        nc.vector.memset(warm_x[:], 0.0)
        # ~9us of warmup: the first weight DMAs take ~9us from issue to
        # consumable, so fill that window with PE activity (also ramps the
        # clock to 8/8 before the first real matmul).
        warm_ps = psum.tile([P, TC], F32, tag="kz", name="warm_ps")
        for k in range(30):
            nc.tensor.matmul(
                warm_ps[:, 0:MN], warm_w[:], warm_x[:], start=True, stop=True
            )

        # Weight + chunk-0 fp8 loads fanned across the 3 DMA-capable rings,
        # in PE consumption order (q0 lands first). The scalar ring's first
        # instruction is its auto ACT-table load, so it gets the last pair.
        # Weights are split into lo/hi column halves as separate tiles: the
        # lo half (output blocks j<4) is half the bytes, so the first real
        # matmul becomes consumable ~2us earlier.
        H2 = H // 2
        rings = [nc.gpsimd, nc.sync, nc.gpsimd, nc.scalar]
        wz_sb = []   # wz_sb[q][half] -> [P, 2, H2] tile
        for q in range(NQ):
            lo = const.tile([P, 2, H2], FP8, tag=f"wz{q}l", name=f"wz_sb{q}l")
            rings[q].dma_start(lo[:], wzT_r[:, 2 * q:2 * q + 2, 0:H2])
            wz_sb.append([lo, None])
        for q in range(NQ):
            x0_q = xpool.tile([P, 2, CHUNKS[0]], FP8, tag=f"x8q{q}",
                              name=f"x8_0_{q}")
            rings[q].dma_start(x0_q[:], x8_r[:, 2 * q:2 * q + 2, 0:CHUNKS[0]])
            x8_tiles[0][q] = x0_q
        for q in range(NQ):
            hi = const.tile([P, 2, H2], FP8, tag=f"wz{q}h", name=f"wz_sb{q}h")
            rings[q].dma_start(hi[:], wzT_r[:, 2 * q:2 * q + 2, H2:H])
            wz_sb[q][1] = hi

        # one DMA for all per-channel consts (5 sections, partition-major)
        cst = const.tile([P, 5 * NH], F32, tag="cst", name="cst")
        nc.sync.dma_start(cst[:], consts.rearrange("(p m) -> p m", m=5 * NH))
        NBZ, BZ, BH, BH5, H0 = 0, 1, 2, 3, 4   # section indices in cst

        def cc(s, j):
            return cst[:, s * NH + j:s * NH + j + 1]

        h0_sb = cst[:, H0 * NH:(H0 + 1) * NH]

        # g(h_0) = max(h_0 + 0.5, sigmoid(h_0)) -> scan seed [P, NH];
        # column j seeds channel block j. (Sigmoid-only: avoids a second
        # ACT table set for Relu.)
        s0 = const.tile([P, NH], F32, tag="s0", name="s0")
        carry = const.tile([P, NH], F32, tag="carry", name="carry")
        nc.scalar.activation(s0[:], h0_sb, AF.Sigmoid)
        nc.vector.scalar_tensor_tensor(
            carry[:], h0_sb, 0.5, s0[:], op0=OP.add, op1=OP.max
        )

        def mm_kz(t, j, tcn):
            kz = psum.tile([P, TC], F32, tag="kz", name=f"kz_{t}_{j}")[:, 0:tcn]
            jc = (j % 4) * P
            for q in range(NQ):
                for m0 in range(0, tcn, MN):
                    m1 = min(m0 + MN, tcn)
                    nc.tensor.matmul(
                        kz[:, m0:m1],
                        wz_sb[q][j // 4][:, :, jc:jc + P],
                        x8_tiles[t][q][:, :, m0:m1],
                        start=(q == 0),
                        stop=(q == NQ - 1),
                        perf_mode=DR,
                    )
            return kz

        def mm_kh(t, j, tcn):
            kh = psum.tile([P, TC], F32, tag="kh", name=f"kh_{t}_{j}")[:, 0:tcn]
            jc = (j % 4) * P
            for q in range(KH8Q):
                for m0 in range(0, tcn, MN):
                    m1 = min(m0 + MN, tcn)
                    nc.tensor.matmul(
                        kh[:, m0:m1],
                        wh8_sb[q][j // 4][:, :, jc:jc + P],
                        x8_tiles[t][q][:, :, m0:m1],
                        start=(q == 0),
                        stop=False,
                        perf_mode=DR,
                    )
            for i in range(NI16):
                for m0 in range(0, tcn, MN):
                    m1 = min(m0 + MN, tcn)
                    nc.tensor.matmul(
                        kh[:, m0:m1],
                        wh_sb[i][:, j * P:(j + 1) * P],
                        x16_tiles[t][i][:, m0:m1],
                        start=False,
                        stop=(i == NI16 - 1),
                    )
            return kh

        def act_az(t, j, tcn, kz, z_on_dve):
            a_t = apool.tile([P, TC], FP16, tag="a", name=f"a_{t}_{j}")[:, 0:tcn]
            z_t = apool.tile([P, TC], FP16, tag="z", name=f"z_{t}_{j}")[:, 0:tcn]
            nc.scalar.activation(
                a_t[:], kz[:], AF.Sigmoid, bias=cc(NBZ, j), scale=-DESCALE_Z,
            )
            if z_on_dve:
                # z = 1 - a from SBUF: frees the kz PSUM bank after the 'a'
                # read alone, so PSUM recycling doesn't pace the PE when kz
                # groups are short (chunk 0 and the small tail chunks).
                nc.vector.tensor_scalar(
                    z_t[:], a_t[:], 1.0, -1.0, op0=OP.subtract, op1=OP.mult
                )
            else:
                nc.scalar.activation(
                    z_t[:], kz[:], AF.Sigmoid, bias=cc(BZ, j), scale=DESCALE_Z,
                )
            return a_t, z_t

        def tail_chain(t, j, tcn, off, a_t, z_t, kh):
            s_t = act.tile([P, TC], FP16, tag="s", name=f"s_{t}_{j}")[:, 0:tcn]
            g_t = act.tile([P, TC], FP16, tag="g", name=f"g_{t}_{j}")[:, 0:tcn]
            v_t = act.tile([P, TC], FP16, tag="v", name=f"v_{t}_{j}")[:, 0:tcn]
            nc.scalar.activation(
                s_t[:], kh[:], AF.Sigmoid, bias=cc(BH, j), scale=1.0
            )
            # g = max(kh + (bh + 0.5), sigmoid(kh + bh)); fp32 PSUM left arg
            nc.vector.scalar_tensor_tensor(
                g_t[:], kh[:], cc(BH5, j), s_t[:], op0=OP.add, op1=OP.max,
            )
            nc.vector.tensor_tensor(v_t[:], z_t[:], g_t[:], op=OP.mult)
            h_t = hpool.tile([P, TC], FP16, tag=f"h{j}", name=f"h_{t}_{j}")[:, 0:tcn]
            h_tiles[t][j] = h_t
            init = carry[:, j:j + 1] if t == 0 else (
                h_tiles[t - 1][j][:, CHUNKS[t - 1] - 1:CHUNKS[t - 1]]
            )
            nc.vector.tensor_tensor_scan(
                h_t[:], a_t[:], v_t[:], init, op0=OP.mult, op1=OP.add
            )
            nc.sync.dma_start(out[j * P:(j + 1) * P, off:off + tcn], h_t[:])

        fetched = {0}

        def prefetch(tn):
            # load chunk tn's x tiles; the short tail chunks get dedicated
            # tags in a deeper pool so they can all be fetched during chunk
            # 3 (they are shorter than the DMA issue->consumable latency)
            if tn >= NT or tn in fetched:
                return
            fetched.add(tn)
            noff = sum(CHUNKS[:tn])
            tcn1 = CHUNKS[tn]
            pool, sml = (xpool, "") if tcn1 == TC else (spool, "s")
            for q in range(NQ):
                xn_q = pool.tile([P, 2, tcn1], FP8, tag=f"x8q{q}{sml}",
                                 name=f"x8_{tn}_{q}")
                nc.gpsimd.dma_start(
                    xn_q[:], x8_r[:, 2 * q:2 * q + 2, noff:noff + tcn1]
                )
                x8_tiles[tn][q] = xn_q
            for i in range(NI16):
                xn_i = pool.tile([P, tcn1], FP16, tag=f"x{i}{sml}",
                                 name=f"x_{tn}_{i}")
                nc.sync.dma_start(xn_i[:], xT_r[:, i, noff:noff + tcn1])
                x16_tiles[tn][i] = xn_i

        # ---- chunk 0: wh + x16 fanned over all 3 rings, then per-j work ----
        tcn0 = CHUNKS[0]
        wh8_sb = []
        for q in range(KH8Q):
            lo = const.tile([P, 2, H2], FP8, tag=f"wh8{q}l", name=f"wh8_sb{q}l")
            nc.scalar.dma_start(lo[:], wh8T_r[:, 2 * q:2 * q + 2, 0:H2])
            hi = const.tile([P, 2, H2], FP8, tag=f"wh8{q}h", name=f"wh8_sb{q}h")
            nc.scalar.dma_start(hi[:], wh8T_r[:, 2 * q:2 * q + 2, H2:H])
            wh8_sb.append([lo, hi])
        wh_sb = [None] * NI16
        rot = [nc.sync, nc.scalar, nc.gpsimd]
        # interleave (wh_i, x0_i) pairs per ring in i order, so low i-tiles
        # become consumable first and the kh matmul groups can start while
        # the high i-tiles are still in flight
        for i in range(NI16):
            wh_i = const.tile([P, H], FP16, tag=f"wh{i}", name=f"wh_sb{i}")
            rot[i % 3].dma_start(wh_i[:], whT_r[:, i, :])
            wh_sb[i] = wh_i
            x0_i = xpool.tile([P, tcn0], FP16, tag=f"x{i}", name=f"x_0_{i}")
            rot[i % 3].dma_start(x0_i[:], xT_r[:, i, 0:tcn0])
            x16_tiles[0][i] = x0_i
        az0 = [None] * NH
        for j in range(NH):
            kz0 = mm_kz(0, j, tcn0)
            az0[j] = act_az(0, j, tcn0, kz0, z_on_dve=True)
        prefetch(1)   # queue chunk-1 loads ahead of any store on the rings
        # Pin chunk-0's k_h so the scheduler keeps every k_z matmul (fed by
        # the small fp8 loads) ahead of the k_h matmuls (which need all of
        # wh + x16) in the PE FIFO.
        with tc.tile_wait_until(0.015):
            for j in range(NH):
                kh = mm_kh(0, j, tcn0)
                tail_chain(0, j, tcn0, 0, az0[j][0], az0[j][1], kh)

        off = tcn0
        for t in range(1, NT):
            tcn = CHUNKS[t]
            for j in range(NH):
                kz = mm_kz(t, j, tcn)
                if j == 0:
                    # loads for chunk t+1 enter the rings before this
                    # chunk's stores (which wait on scans) can block them
                    prefetch(t + 1)
                if t == 3 and j == 4:
                    # the final short chunk is shorter than the DMA
                    # latency: fetch it one chunk early
                    prefetch(5)
                a_t, z_t = act_az(t, j, tcn, kz, z_on_dve=(tcn != TC))
                kh = mm_kh(t, j, tcn)
                tail_chain(t, j, tcn, off, a_t, z_t, kh)
            off += tcn

    nc.compile()
    return nc


def _get_program():
    global _PROGRAM
    if _PROGRAM is None:
        _PROGRAM = _build_program()
    return _PROGRAM


def _make_in_maps(x, h_0, W_z, b_z, W_h, b_h):
    def pmajor(v):
        # [NH*P] channel-major -> partition-major so the SBUF-side [P, NH]
        # tile DMA reads one contiguous segment per partition.
        return np.ascontiguousarray(
            v.astype(np.float32).reshape(NH, P).T.reshape(-1)
        )

    n16 = 2 * KH8Q * P  # first rows of the contraction handled in fp8
    wzT = np.ascontiguousarray((W_z.T * WZS).astype(FP8_NP))
    wh8T = np.ascontiguousarray((W_h.T[:n16] * WHS).astype(FP8_NP))
    whT = np.ascontiguousarray(W_h.T[n16:].astype(np.float16))
    shared = [pmajor(-b_z), pmajor(b_z), pmajor(b_h), pmajor(b_h + 0.5)]
    in_maps = []
    for b in range(B):
        xTb = x[b].T
        # packed consts, contiguous per partition: [P, 5*NH] row-major
        sec = np.stack([v.reshape(P, NH) for v in
                        shared + [pmajor(h_0[b])]], axis=1)  # [P, 5, NH]
        in_maps.append({
            "xT": np.ascontiguousarray(xTb[n16:].astype(np.float16)),
            "x8": np.ascontiguousarray((xTb * XS).astype(FP8_NP)),
            "wzT": wzT,
            "wh8T": wh8T,
            "whT": whT,
            "consts": np.ascontiguousarray(sec.reshape(-1).astype(np.float32)),
        })
    return in_maps


def _run(x, h_0, W_z, b_z, W_h, b_h, trace=False):
    x, h_0, W_z, b_z, W_h, b_h = (
        np.asarray(a) for a in (x, h_0, W_z, b_z, W_h, b_h)
    )
    nc = _get_program()
    in_maps = _make_in_maps(x, h_0, W_z, b_z, W_h, b_h)
    res = run_bass_kernel_spmd(nc, in_maps, core_ids=list(range(B)), trace=trace)
    out = np.stack(
        [res.results[b]["out"].T.astype(np.float32) for b in range(B)], axis=0
    )
    return out, res


def kernel(x, h_0, W_z, b_z, W_h, b_h):
    out, _ = _run(x, h_0, W_z, b_z, W_h, b_h)
    return out


# revision 40
# speedup vs baseline: 1.0259x; 1.0259x over previous
"""MinGRU on Trainium2 (Bass/Tile), data-parallel over batch on 8 NeuronCores.

Math (per batch element, per hidden channel):
    k_z = x @ W_z.T + b_z
    k_h = x @ W_h.T + b_h
    a   = sigmoid(-k_z)                  # = exp(log_coeffs)
    z   = sigmoid(k_z) = 1 - a
    g(u)= u + 0.5 if u >= 0 else sigmoid(u)
        = max(u + 0.5, sigmoid(u))       # exact: sigmoid(u)-(u+0.5) >=0 iff u<=0
    v   = z * g(k_h)
    h_t = a_t * h_{t-1} + v_t,  h_init = g(h_0)        (t = 1..T)
Output is h_1..h_T, shape [B, T, H].

Device layout: each core gets one batch element. Hidden dim H on SBUF
partitions (8 tiles of 128), time T on the free dim; the recurrence is the
DVE TensorTensorScan instruction (fp32 state).

Precision (validated by CPU emulation, rel_err 1.57e-2 vs 2e-2 gate):
  - One fp8(e4m3) x stream at x/8 scale feeds BOTH matmuls' fp8 parts
    (fp8 products need consistent PSUM scale; w carries the complement).
  - k_z: all 4 DoubleRow pairs fp8 (w = 4096*W_z, descale 2^-9 folded into
    the ACT scale). k_z errors reach h only through sigmoid' <= 1/4.
  - k_h: 1 DoubleRow pair fp8 (rows 0:255, w = 8*W_h, true scale) + 6 fp16
    i-tiles (rows 256:1023). g has slope 1, so k_h tolerates only ~1/4 of
    the contraction in fp8. fp16 tiles (not bf16) keep the pipeline's own
    rounding noise negligible.
  - Tail per tile: ACT a = sig(-2^-9 kz - bz), z = sig(+2^-9 kz + bz),
    s = sig(kh + bh) (3 ACT ops from PSUM); DVE g = (kh +(bh+.5)) max s
    (one STT from PSUM), v = z*g (2x TT), h = scan(a, v). Everything fp16.
  - PE/tile(1024): kz 8 MM + kh 14 MM = 22 x 216ns -> ~152us busy; DVE
    ~128us, ACT ~92us fit underneath.

DMA: weight pair loads split across 4 rings (gpsimd/sync/scalar/vector) so
the first k_z matmul waits on ~2 transfers instead of a serial ring. x8
chunks on gpsimd, x16 on sync, stores alternate sync/gpsimd. Chunks
[512,1024,1024,1024,256,256]: small head chunk to start the PE early,
small tail chunks so the last scan+store tail is short.
"""

import numpy as np
from contextlib import ExitStack

import concourse.bass as bass
import concourse.tile as tile
from concourse import bacc, mybir
from concourse.bass_utils import run_bass_kernel_spmd

B, T, I, H = 8, 4096, 1024, 1024
P = 128           # SBUF partitions
TC = 1024         # max T chunk (2 PSUM banks of fp32)
MN = 512          # matmul moving free dim (one PSUM bank)
CHUNKS = [512, 1024, 1024, 1024, 256, 256]
assert sum(CHUNKS) == T
NI, NH = I // P, H // P
NQ = NI // 2      # fp8 DoubleRow contraction pairs (kz)
KH8Q = 1          # kh fp8 DoubleRow pairs (rows 0 : KH8Q*256)
NI16 = NI - 2 * KH8Q   # kh fp16 i-tiles (rows KH8Q*256 : 1024)
NT = len(CHUNKS)
F32 = mybir.dt.float32
FP16 = mybir.dt.float16
FP8 = mybir.dt.float8e4
import ml_dtypes
FP8_NP = ml_dtypes.float8_e4m3
XS = 0.125        # host-side fp8 scale on x (shared by kz and kh fp8 parts)
WZS = 4096.0      # host-side fp8 scale on W_z
WHS = 8.0         # host-side fp8 scale on W_h pair (XS*WHS = 1: true scale)
DESCALE_Z = 1.0 / (XS * WZS)  # 2^-9
AF = mybir.ActivationFunctionType
OP = mybir.AluOpType
DR = mybir.MatmulPerfMode.DoubleRow

_PROGRAM = None


def _build_program():
    nc = bacc.Bacc("TRN2", target_bir_lowering=False, debug=False)
    xT = nc.dram_tensor("xT", [NI16 * P, T], FP16, kind="ExternalInput").ap()
    x8 = nc.dram_tensor("x8", [I, T], FP8, kind="ExternalInput").ap()
    wzT = nc.dram_tensor("wzT", [I, H], FP8, kind="ExternalInput").ap()
    wh8T = nc.dram_tensor("wh8T", [2 * KH8Q * P, H], FP8,
                          kind="ExternalInput").ap()
    whT = nc.dram_tensor("whT", [NI16 * P, H], FP16, kind="ExternalInput").ap()
    # packed per-partition-contiguous consts: [-b_z, b_z, b_h, b_h+0.5, h_0]
    consts = nc.dram_tensor("consts", [5 * H], F32, kind="ExternalInput").ap()
    out = nc.dram_tensor("out", [H, T], FP16, kind="ExternalOutput").ap()

    with tile.TileContext(nc) as tc, ExitStack() as ctx:
        const = ctx.enter_context(tc.tile_pool(name="const", bufs=1))
        xpool = ctx.enter_context(tc.tile_pool(name="xp", bufs=2))
        spool = ctx.enter_context(tc.tile_pool(name="xs", bufs=3))
        psum = ctx.enter_context(tc.tile_pool(name="ps", bufs=2, space="PSUM"))
        apool = ctx.enter_context(tc.tile_pool(name="ap", bufs=10))
        act = ctx.enter_context(tc.tile_pool(name="actp", bufs=4))
        hpool = ctx.enter_context(tc.tile_pool(name="hp", bufs=2))

        wzT_r = wzT.rearrange("(k p) h -> p k h", p=P)
        wh8T_r = wh8T.rearrange("(k p) h -> p k h", p=P)
        whT_r = whT.rearrange("(n p) h -> p n h", p=P)
        xT_r = xT.rearrange("(n p) t -> p n t", p=P)
        x8_r = x8.rearrange("(k p) t -> p k t", p=P)

        x8_tiles = [[None] * NQ for _ in range(NT)]
        x16_tiles = [[None] * NI16 for _ in range(NT)]
        h_tiles = [[None] * NH for _ in range(NT)]

        # PE warmup: throwaway matmuls on memset tiles while the first DMAs
        # land; ~4us of sustained PE activity moves the HAM clock-gate from
        # 4/8 to 8/8 so the first real matmuls run at 2.4 GHz.
        warm_w = const.tile([P, P], FP16, tag="warmw", name="warm_w")
        warm_x = const.tile([P, MN], FP16, tag="warmx", name="warm_x")
        nc.vector.memset(warm_w[:], 0.0)
        nc.vector.memset(warm_x[:], 0.0)
        # ~9us of warmup: the first weight DMAs take ~9us from issue to
        # consumable, so fill that window with PE activity (also ramps the
        # clock to 8/8 before the first real matmul).
        warm_ps = psum.tile([P, TC], F32, tag="kz", name="warm_ps")
        for k in range(30):
            nc.tensor.matmul(
                warm_ps[:, 0:MN], warm_w[:], warm_x[:], start=True, stop=True
            )

        # Weight + chunk-0 fp8 loads fanned across the 3 DMA-capable rings,
        # in PE consumption order (q0 lands first). The scalar ring's first
        # instruction is its auto ACT-table load, so it gets the last pair.
        # Weights are split into lo/hi column halves as separate tiles: the
        # lo half (output blocks j<4) is half the bytes, so the first real
        # matmul becomes consumable ~2us earlier.
        H2 = H // 2
        rings = [nc.gpsimd, nc.sync, nc.gpsimd, nc.scalar]
        wz_sb = []   # wz_sb[q][half] -> [P, 2, H2] tile
        for q in range(NQ):
            lo = const.tile([P, 2, H2], FP8, tag=f"wz{q}l", name=f"wz_sb{q}l")
            rings[q].dma_start(lo[:], wzT_r[:, 2 * q:2 * q + 2, 0:H2])
            wz_sb.append([lo, None])
        for q in range(NQ):
            x0_q = xpool.tile([P, 2, CHUNKS[0]], FP8, tag=f"x8q{q}",
                              name=f"x8_0_{q}")
            rings[q].dma_start(x0_q[:], x8_r[:, 2 * q:2 * q + 2, 0:CHUNKS[0]])
            x8_tiles[0][q] = x0_q
        for q in range(NQ):
            hi = const.tile([P, 2, H2], FP8, tag=f"wz{q}h", name=f"wz_sb{q}h")
            rings[q].dma_start(hi[:], wzT_r[:, 2 * q:2 * q + 2, H2:H])
            wz_sb[q][1] = hi

        # one DMA for all per-channel consts (5 sections, partition-major)
        cst = const.tile([P, 5 * NH], F32, tag="cst", name="cst")
        nc.sync.dma_start(cst[:], consts.rearrange("(p m) -> p m", m=5 * NH))
        NBZ, BZ, BH, BH5, H0 = 0, 1, 2, 3, 4   # section indices in cst

        def cc(s, j):
            return cst[:, s * NH + j:s * NH + j + 1]

        h0_sb = cst[:, H0 * NH:(H0 + 1) * NH]

        # g(h_0) = max(h_0 + 0.5, sigmoid(h_0)) -> scan seed [P, NH];
        # column j seeds channel block j. (Sigmoid-only: avoids a second
        # ACT table set for Relu.)
        s0 = const.tile([P, NH], F32, tag="s0", name="s0")
        carry = const.tile([P, NH], F32, tag="carry", name="carry")
        nc.scalar.activation(s0[:], h0_sb, AF.Sigmoid)
        nc.vector.scalar_tensor_tensor(
            carry[:], h0_sb, 0.5, s0[:], op0=OP.add, op1=OP.max
        )

        def mm_kz(t, j, tcn):
            kz = psum.tile([P, TC], F32, tag="kz", name=f"kz_{t}_{j}")[:, 0:tcn]
            jc = (j % 4) * P
            for q in range(NQ):
                for m0 in range(0, tcn, MN):
                    m1 = min(m0 + MN, tcn)
                    nc.tensor.matmul(
                        kz[:, m0:m1],
                        wz_sb[q][j // 4][:, :, jc:jc + P],
                        x8_tiles[t][q][:, :, m0:m1],
                        start=(q == 0),
                        stop=(q == NQ - 1),
                        perf_mode=DR,
                    )
            return kz

        def mm_kh(t, j, tcn):
            kh = psum.tile([P, TC], F32, tag="kh", name=f"kh_{t}_{j}")[:, 0:tcn]
            jc = (j % 4) * P
            for q in range(KH8Q):
                for m0 in range(0, tcn, MN):
                    m1 = min(m0 + MN, tcn)
                    nc.tensor.matmul(
                        kh[:, m0:m1],
                        wh8_sb[q][j // 4][:, :, jc:jc + P],
                        x8_tiles[t][q][:, :, m0:m1],
                        start=(q == 0),
                        stop=False,
                        perf_mode=DR,
                    )
            for i in range(NI16):
                for m0 in range(0, tcn, MN):
                    m1 = min(m0 + MN, tcn)
                    nc.tensor.matmul(
                        kh[:, m0:m1],
                        wh_sb[i][j // 4][:, jc:jc + P],
                        x16_tiles[t][i][:, m0:m1],
                        start=False,
                        stop=(i == NI16 - 1),
                    )
            return kh

        def act_az(t, j, tcn, kz, z_on_dve):
            a_t = apool.tile([P, TC], FP16, tag="a", name=f"a_{t}_{j}")[:, 0:tcn]
            z_t = apool.tile([P, TC], FP16, tag="z", name=f"z_{t}_{j}")[:, 0:tcn]
            nc.scalar.activation(
                a_t[:], kz[:], AF.Sigmoid, bias=cc(NBZ, j), scale=-DESCALE_Z,
            )
            if z_on_dve:
                # z = 1 - a from SBUF: frees the kz PSUM bank after the 'a'
                # read alone, so PSUM recycling doesn't pace the PE when kz
                # groups are short (chunk 0 and the small tail chunks).
                nc.vector.tensor_scalar(
                    z_t[:], a_t[:], 1.0, -1.0, op0=OP.subtract, op1=OP.mult
                )
            else:
                nc.scalar.activation(
                    z_t[:], kz[:], AF.Sigmoid, bias=cc(BZ, j), scale=DESCALE_Z,
                )
            return a_t, z_t

        def tail_chain(t, j, tcn, off, a_t, z_t, kh):
            s_t = act.tile([P, TC], FP16, tag="s", name=f"s_{t}_{j}")[:, 0:tcn]
            g_t = act.tile([P, TC], FP16, tag="g", name=f"g_{t}_{j}")[:, 0:tcn]
            v_t = act.tile([P, TC], FP16, tag="v", name=f"v_{t}_{j}")[:, 0:tcn]
            nc.scalar.activation(
                s_t[:], kh[:], AF.Sigmoid, bias=cc(BH, j), scale=1.0
            )
            # g = max(kh + (bh + 0.5), sigmoid(kh + bh)); fp32 PSUM left arg
            nc.vector.scalar_tensor_tensor(
                g_t[:], kh[:], cc(BH5, j), s_t[:], op0=OP.add, op1=OP.max,
            )
            nc.vector.tensor_tensor(v_t[:], z_t[:], g_t[:], op=OP.mult)
            h_t = hpool.tile([P, TC], FP16, tag=f"h{j}", name=f"h_{t}_{j}")[:, 0:tcn]
            h_tiles[t][j] = h_t
            init = carry[:, j:j + 1] if t == 0 else (
                h_tiles[t - 1][j][:, CHUNKS[t - 1] - 1:CHUNKS[t - 1]]
            )
            nc.vector.tensor_tensor_scan(
                h_t[:], a_t[:], v_t[:], init, op0=OP.mult, op1=OP.add
            )
            nc.sync.dma_start(out[j * P:(j + 1) * P, off:off + tcn], h_t[:])

        fetched = {0}

        def prefetch(tn):
            # load chunk tn's x tiles; the short tail chunks get dedicated
            # tags in a deeper pool so they can all be fetched during chunk
            # 3 (they are shorter than the DMA issue->consumable latency)
            if tn >= NT or tn in fetched:
                return
            fetched.add(tn)
            noff = sum(CHUNKS[:tn])
            tcn1 = CHUNKS[tn]
            pool, sml = (xpool, "") if tcn1 == TC else (spool, "s")
            for q in range(NQ):
                xn_q = pool.tile([P, 2, tcn1], FP8, tag=f"x8q{q}{sml}",
                                 name=f"x8_{tn}_{q}")
                nc.gpsimd.dma_start(
                    xn_q[:], x8_r[:, 2 * q:2 * q + 2, noff:noff + tcn1]
                )
                x8_tiles[tn][q] = xn_q
            for i in range(NI16):
                xn_i = pool.tile([P, tcn1], FP16, tag=f"x{i}{sml}",
                                 name=f"x_{tn}_{i}")
                nc.sync.dma_start(xn_i[:], xT_r[:, i, noff:noff + tcn1])
                x16_tiles[tn][i] = xn_i

        # ---- chunk 0: wh + x16 fanned over all 3 rings, then per-j work ----
        tcn0 = CHUNKS[0]
        wh8_sb = []
        for q in range(KH8Q):
            lo = const.tile([P, 2, H2], FP8, tag=f"wh8{q}l", name=f"wh8_sb{q}l")
            nc.scalar.dma_start(lo[:], wh8T_r[:, 2 * q:2 * q + 2, 0:H2])
            wh8_sb.append([lo, None])
        wh_sb = [None] * NI16
        rot = [nc.sync, nc.scalar, nc.gpsimd]
        # interleave (wh_i lo-half, x0_i) pairs per ring in i order, so the
        # inputs of kh output blocks j<4 all become consumable first and the
        # chunk-0 kh matmuls start ~3us earlier; hi halves stream afterwards
        for i in range(NI16):
            lo = const.tile([P, H2], FP16, tag=f"wh{i}l", name=f"wh_sb{i}l")
            rot[i % 3].dma_start(lo[:], whT_r[:, i, 0:H2])
            wh_sb[i] = [lo, None]
            x0_i = xpool.tile([P, tcn0], FP16, tag=f"x{i}", name=f"x_0_{i}")
            rot[i % 3].dma_start(x0_i[:], xT_r[:, i, 0:tcn0])
            x16_tiles[0][i] = x0_i
        for q in range(KH8Q):
            hi = const.tile([P, 2, H2], FP8, tag=f"wh8{q}h", name=f"wh8_sb{q}h")
            nc.scalar.dma_start(hi[:], wh8T_r[:, 2 * q:2 * q + 2, H2:H])
            wh8_sb[q][1] = hi
        for i in range(NI16):
            hi = const.tile([P, H2], FP16, tag=f"wh{i}h", name=f"wh_sb{i}h")
            rot[i % 3].dma_start(hi[:], whT_r[:, i, H2:H])
            wh_sb[i][1] = hi
        az0 = [None] * NH
        for j in range(NH):
            kz0 = mm_kz(0, j, tcn0)
            az0[j] = act_az(0, j, tcn0, kz0, z_on_dve=True)
        prefetch(1)   # queue chunk-1 loads ahead of any store on the rings
        # Pin chunk-0's k_h so the scheduler keeps every k_z matmul (fed by
        # the small fp8 loads) ahead of the k_h matmuls (which need all of
        # wh + x16) in the PE FIFO.
        with tc.tile_wait_until(0.015):
            for j in range(NH):
                kh = mm_kh(0, j, tcn0)
                tail_chain(0, j, tcn0, 0, az0[j][0], az0[j][1], kh)

        off = tcn0
        for t in range(1, NT):
            tcn = CHUNKS[t]
            for j in range(NH):
                kz = mm_kz(t, j, tcn)
                if j == 0:
                    # loads for chunk t+1 enter the rings before this
                    # chunk's stores (which wait on scans) can block them
                    prefetch(t + 1)
                if t == 3 and j == 4:
                    # the final short chunk is shorter than the DMA
                    # latency: fetch it one chunk early
                    prefetch(5)
                a_t, z_t = act_az(t, j, tcn, kz, z_on_dve=(tcn != TC))
                kh = mm_kh(t, j, tcn)
                tail_chain(t, j, tcn, off, a_t, z_t, kh)
            off += tcn

    nc.compile()
    return nc


def _get_program():
    global _PROGRAM
    if _PROGRAM is None:
        _PROGRAM = _build_program()
    return _PROGRAM


def _make_in_maps(x, h_0, W_z, b_z, W_h, b_h):
    def pmajor(v):
        # [NH*P] channel-major -> partition-major so the SBUF-side [P, NH]
        # tile DMA reads one contiguous segment per partition.
        return np.ascontiguousarray(
            v.astype(np.float32).reshape(NH, P).T.reshape(-1)
        )

    n16 = 2 * KH8Q * P  # first rows of the contraction handled in fp8
    wzT = np.ascontiguousarray((W_z.T * WZS).astype(FP8_NP))
    wh8T = np.ascontiguousarray((W_h.T[:n16] * WHS).astype(FP8_NP))
    whT = np.ascontiguousarray(W_h.T[n16:].astype(np.float16))
    shared = [pmajor(-b_z), pmajor(b_z), pmajor(b_h), pmajor(b_h + 0.5)]
    in_maps = []
    for b in range(B):
        xTb = x[b].T
        # packed consts, contiguous per partition: [P, 5*NH] row-major
        sec = np.stack([v.reshape(P, NH) for v in
                        shared + [pmajor(h_0[b])]], axis=1)  # [P, 5, NH]
        in_maps.append({
            "xT": np.ascontiguousarray(xTb[n16:].astype(np.float16)),
            "x8": np.ascontiguousarray((xTb * XS).astype(FP8_NP)),
            "wzT": wzT,
            "wh8T": wh8T,
            "whT": whT,
            "consts": np.ascontiguousarray(sec.reshape(-1).astype(np.float32)),
        })
    return in_maps


def _run(x, h_0, W_z, b_z, W_h, b_h, trace=False):
    x, h_0, W_z, b_z, W_h, b_h = (
        np.asarray(a) for a in (x, h_0, W_z, b_z, W_h, b_h)
    )
    nc = _get_program()
    in_maps = _make_in_maps(x, h_0, W_z, b_z, W_h, b_h)
    res = run_bass_kernel_spmd(nc, in_maps, core_ids=list(range(B)), trace=trace)
    out = np.stack(
        [res.results[b]["out"].T.astype(np.float32) for b in range(B)], axis=0
    )
    return out, res


def kernel(x, h_0, W_z, b_z, W_h, b_h):
    out, _ = _run(x, h_0, W_z, b_z, W_h, b_h)
    return out
